# revision 15
# baseline (speedup 1.0000x reference)
"""AttentivePooling Trainium2 kernel.

Computes, per example b:
    h      = tanh(x[b] @ W1 + b1)          # (S, MID)
    scores = h @ w2 (+ b2, dropped: softmax-invariant)
    attn   = softmax(scores)               # over S
    out[b] = attn @ x[b]                   # (C,)

Sharding: batch (32) split across 8 NeuronCores -> 4 examples/core.
Weights replicated. No cross-core communication.

Per-core dataflow (on-chip tensors bf16, accumulation fp32):
  - x loaded HBM->SBUF once in 1MB chunks, fp32->bf16 cast during DMA
    (SWDGE; the only casting path), layout [s=128 partitions, (tile, c)
    free] with seq tiles relabeled so every descriptor covers 4
    contiguous HBM rows. Constants ride the Sync-engine HWDGE ring (W1/
    w2 host-pre-cast to bf16 since HWDGE cannot cast) so gpsimd emits
    x-chunk descriptors from t~0; the priming chunk is split into 4
    sub-loads so the first transposes start after ~256KB. Chunk size is
    a measured optimum: 512KB chunks cost +27us (per-DMA overheads),
    2MB chunks cost +5us and jitter.
  - A burst of 16 dummy N=512 matmuls on memset constants at t~0
    (while the first x chunk is in flight) trips the PE HAM clock gate
    out of its cold 1.2 GHz state before real matmuls arrive.
  - xT via TensorE transpose-mode matmuls ([128,128] bf16 blocks,
    identity moving) into bf16 PSUM tiles, drained to SBUF by VectorE
    copies. (The DMA xbar transpose path is unusable here: Tile globally
    serializes xbar transposes against normal DMAs, and the XPOSE ISA
    slot only carries one semaphore wait -- walrus rejects the 2-3 waits
    a pipelined kernel needs. fp32 pair-view transposes halve the
    instruction count but stream at 2 cycles/row and lose FWL on the
    weight load -- measured a wash, with fp32-mode pipeline pollution on
    neighboring matmuls. fp8 for the scores path fails the 2e-2 gate:
    rel err 2.3e-2 measured.)
  - mm1: hT[m, s] = W1k0.T @ xT[...,half0] + W1k1.T @ xT[...,half1] (PE).
    (Reordering to k0,k0,k1,k1 across two PSUM banks so consecutive
    matmuls share a stationary measured neutral -- the backend does NOT
    dedupe repeat LDWEIGHTS.)
  - tanh(. + b1) via ScalarE per-partition bias, bf16 out.
  - mm2: scoresT[:, jj] = h_chunk.T @ w2 -> scores in [128, 64] layout
    (seq position on partitions, seq tile as column) feeding both the
    softmax and mm4's stationary operand with no reshape.
  - softmax WITHOUT max subtraction: |scores| <= ||w2||_1 + |b2| ~ 5.7,
    exp is safe in fp32. exp via ScalarE with fused row-sum accum_out;
    cross-partition total via a ones-vector matmul; normalization
    deferred to the end (scale by 1/sum).
  - mm4: acc[strip t%4] += p[:, t].T @ x_tile over 64 seq tiles, with
    tile_position=(0, 32*(t%4)) col-group tiling: four M=1 matmuls with
    independent stationaries stream CONCURRENTLY through disjoint 32-col
    array strips (the per-tile LDWEIGHTS+stream no longer serializes;
    measured mm4 26.5us -> 15.2us). The four strip rows live at
    partitions {0,32,64,96} of a DEDICATED PSUM bank in ONE accumulation
    chain -- start=True clears has_written bank-wide, so per-strip
    chains would destroy each other; first-touch of rows 32/64/96 with
    start=False overwrites by has_written semantics (DVE memsets keep
    CoreSim's model in agreement). DVE sums the strips at example
    finish. Weighted sums are interleaved into the NEXT example's score
    phase so the in-order PE never stalls at example boundaries; the
    LAST example instead weaves its mm4s into its OWN chunks via
    per-chunk exp of the just-finished score columns (kills the
    end-of-kernel serial tail).

Measured: 133.8us (previous session) -> ~122.6us (this session; 122.5/
122.7 back-to-back in a healthy device window), rel err 2.19e-3,
DMA-roofline-dominated (HBM read of fp32 x: ~94us min; DMA engines
active ~104us; Tensor busy ~103us). The chip intermittently enters a
~15%-slower sustained-load power state (~145us) that affects any build
equally.
"""

from contextlib import ExitStack

import numpy as np

import concourse.bass as bass
import concourse.tile as tile
from concourse import mybir
from concourse.bass_utils import run_bass_kernel_spmd

B, S, C, MID = 32, 8192, 256, 128
N_CORES = 8
B_LOC = B // N_CORES  # 4 examples per core

F32 = mybir.dt.float32
BF16 = mybir.dt.bfloat16
AF = mybir.ActivationFunctionType

S_TILES = S // 128  # 64 seq tiles of 128 per example
CH = 8  # chunks per example (1024 seq each; measured optimum -- both
# halving (CH=16) and doubling (CH=4) the DMA size cost 3-27us)
T_PER_CH = S_TILES // CH  # 8 seq tiles per chunk


def build_nc(reps: int = 1, strip_waits: bool = True) -> bass.Bass:
    """Build the per-core program. reps>1 repeats the whole computation
    back-to-back inside one NEFF (used only for benchmarking: the wall-time
    difference between reps=R and reps=1 isolates kernel time from dispatch
    overhead). strip_waits=False keeps Tile's full (redundant) semaphore
    waits -- required for CoreSim, whose race detector doesn't model
    engine-FIFO-implied ordering; hardware builds need the strip because
    walrus allows only one sync wait per engine ISA instruction."""
    nc = bass.Bass("TRN2", target_bir_lowering=False, debug=False)

    x_ext = nc.declare_dram_parameter("x", [B_LOC, S, C], F32, isOutput=False)
    # W1/w2 arrive host-pre-cast to bf16 (tiny constants) so their loads can
    # ride the Sync HWDGE ring -- only gpsimd DMAs can cast, and gpsimd must
    # stay free for the x-chunk pipeline from t=0.
    w1_ext = nc.declare_dram_parameter("W1", [C, MID], BF16, isOutput=False)
    b1_ext = nc.declare_dram_parameter("b1", [MID], F32, isOutput=False)
    w2_ext = nc.declare_dram_parameter("w2", [MID], BF16, isOutput=False)
    id_ext = nc.declare_dram_parameter("ident", [128, 128], BF16, isOutput=False)
    out_ext = nc.declare_dram_parameter("out", [B_LOC, C], F32, isOutput=True)

    with tile.TileContext(nc) as tc, ExitStack() as ctx:
        const_pool = ctx.enter_context(tc.tile_pool(name="const", bufs=1))
        # All 32 chunks resident (128KB/partition): no slot reuse means no
        # WAW pacing waits in the DMA queues -- the whole x load streams
        # back-to-back, so the E64 engine (which also serves the ~8us of
        # instruction-fetch DMAs) drains its share continuously instead of
        # crawling through a lone-engine backlog at the end of the load.
        xnat_pool = ctx.enter_context(tc.tile_pool(name="xnat", bufs=4 * CH))
        xt_pool = ctx.enter_context(tc.tile_pool(name="xt", bufs=3))
        ht_pool = ctx.enter_context(tc.tile_pool(name="ht", bufs=4))
        small_pool = ctx.enter_context(tc.tile_pool(name="small", bufs=4))
        psum_xt = ctx.enter_context(tc.tile_pool(name="psum_xt", bufs=1, space="PSUM"))
        psum_ht = ctx.enter_context(tc.tile_pool(name="psum_ht", bufs=2, space="PSUM"))
        psum_sc = ctx.enter_context(tc.tile_pool(name="psum_sc", bufs=2, space="PSUM"))
        psum_oa = ctx.enter_context(tc.tile_pool(name="psum_oa", bufs=2, space="PSUM"))

        # ---- HAM warm-up burst (no DMA dependencies: memset-fed) ----
        ones_bf = const_pool.tile([128, 1], BF16, tag="ones_bf")
        nc.vector.memset(ones_bf[:], 1.0)
        ones_sb = const_pool.tile([128, 1], F32, tag="ones")
        nc.vector.memset(ones_sb[:], 1.0)
        wburst_sb = const_pool.tile([128, 512], BF16, tag="wburst")
        nc.vector.memset(wburst_sb[:], 1.0)
        warm_ps = psum_ht.tile([128, 512], F32, tag="hT")
        for _ in range(16):
            nc.tensor.matmul(
                warm_ps[0:1, :], ones_bf[:], wburst_sb[:], start=True, stop=True
            )

        # ---- constants (one-time, tiny; Sync-engine HWDGE ring so they
        # don't queue behind x-chunk SWDGE descriptor generation) ----
        w1_sb = const_pool.tile([128, 2, MID], BF16, tag="w1")
        nc.sync.dma_start(
            out=w1_sb[:], in_=w1_ext[:].rearrange("(k p) m -> p k m", p=128)
        )
        w2_sb = const_pool.tile([128, 1], BF16, tag="w2")
        nc.sync.dma_start(out=w2_sb[:], in_=w2_ext[:].rearrange("(p o) -> p o", o=1))
        b1_sb = const_pool.tile([128, 1], F32, tag="b1")
        nc.sync.dma_start(out=b1_sb[:], in_=b1_ext[:].rearrange("(p o) -> p o", o=1))
        ident_sb = const_pool.tile([128, 128], BF16, tag="ident")
        nc.sync.dma_start(out=ident_sb[:], in_=id_ext[:])
        # Warm up the ScalarE activation table (exp_and_others: tanh+exp) on a
        # tiny input. Walrus attaches the table-load sync to the first
        # ACTIVATE; without this, that instruction exceeds the ISA's
        # 2-sync-wait budget once Tile's own deps are added.
        warm_sb = const_pool.tile([128, 1], F32, tag="warm")
        nc.scalar.activation(warm_sb[:], b1_sb[:], AF.Tanh, bias=b1_sb[:])
        # Warm the other engines too, and -- crucially -- make the PE observe
        # every constant's DMA-lane semaphore via one-wait warmup matmuls, so
        # no steady-state matmul ever needs a second (constant-load) wait.
        # Walrus enforces at most ONE sync wait per engine ISA instruction.
        warm2_sb = const_pool.tile([128, 1], F32, tag="warm2")
        nc.vector.tensor_copy(warm2_sb[:], ones_sb[:])
        nc.tensor.matmul(
            warm_ps[0:1, 0:1], ones_sb[:], ones_sb[:], start=True, stop=True
        )
        nc.tensor.matmul(
            warm_ps[0:1, 0:1], ident_sb[:, 0:1], ones_bf[:], start=True, stop=True
        )
        nc.tensor.matmul(
            warm_ps[0:1, 0:1], w1_sb[:, 0, 0:1], ones_bf[:], start=True, stop=True
        )
        nc.tensor.matmul(
            warm_ps[0:1, 0:1], ones_bf[:], w2_sb[:], start=True, stop=True
        )

        def emit_mm4_slice(st, lo, hi):
            """Accumulate seq tiles [lo, hi) of a pending example into the
            4 col-group strip accumulators (partitions 0/32/64/96)."""
            acc = st["acc"]
            for t in range(lo, hi):
                j = t % 4
                # skip_group_check: CoreSim's zero-region tracker flattens
                # base_partition!=0 PSUM APs to bogus (partition, bank) keys,
                # spuriously colliding with other pools. The strips write
                # disjoint partitions {0,32,64,96} of one bank; per-element
                # has_written accumulation is genuinely race-free.
                nc.tensor.matmul(
                    acc[32 * j : 32 * j + 1, :],
                    st["p_bf"][:, t : t + 1],
                    st["x_chunks"][t // T_PER_CH][:, t % T_PER_CH, :],
                    start=(t == 0),
                    stop=(t == S_TILES - 1),
                    tile_position=(0, 32 * j),
                    skip_group_check=True,
                )

        def finish_phase_b(st):
            # DVE reads at most ONE non-scalar operand from PSUM per
            # instruction (NCC_IBVF027): chain the strip adds through SBUF.
            acc = st["acc"]
            t0 = small_pool.tile([1, C], F32, tag="t0")
            nc.vector.tensor_copy(t0[:], acc[0:1, :])
            t1 = small_pool.tile([1, C], F32, tag="t1")
            nc.vector.tensor_tensor(t1[:], t0[:], acc[32:33, :], mybir.AluOpType.add)
            t2 = small_pool.tile([1, C], F32, tag="t2")
            nc.vector.tensor_tensor(t2[:], t1[:], acc[64:65, :], mybir.AluOpType.add)
            tsum = small_pool.tile([1, C], F32, tag="tsum")
            nc.vector.tensor_tensor(
                tsum[:], t2[:], acc[96:97, :], mybir.AluOpType.add
            )
            out_sb = small_pool.tile([1, C], F32, tag="out_sb")
            nc.vector.tensor_scalar_mul(out_sb[:], tsum[:], st["recip"][:])
            nc.scalar.dma_start(out=out_ext[st["b"] : st["b"] + 1, :], in_=out_sb[:])

        pending = None  # example whose weighted-sum mm4s are being woven
        blist = [bb for _ in range(reps) for bb in range(B_LOC)]
        for bi, b in enumerate(blist):
            is_last = bi == len(blist) - 1
            x_chunks = []
            # scoresT cols [0:64]; col [64] holds the cross-partition exp-sum.
            scoresT_ps = psum_sc.tile([128, S_TILES + 1], F32, tag="scoresT")
            # Dedicated accumulator bank: 4 col-group strip rows at partitions
            # {0,32,64,96}, ONE accumulation chain (start only at t=0 -- any
            # start=True clears has_written for the WHOLE bank, so per-strip
            # chains in a shared bank would destroy each other). Rows 32/64/96
            # are first written with start=False (has_written clear -> HW
            # overwrites); the DVE memsets below make CoreSim's accumulate-
            # based model agree with that overwrite semantics.
            acc_ps = psum_oa.tile([128, C], F32, tag="outacc")
            for j in (32, 64, 96):
                nc.vector.memset(acc_ps[j : j + 1, :], 0.0)
            if is_last:
                p_bf = small_pool.tile([128, S_TILES], BF16, tag="p")
                last_st = {"acc": acc_ps, "x_chunks": x_chunks, "b": b,
                           "p_bf": p_bf}
            for ch in range(CH):
                # keep the PE busy across example boundaries: weave the
                # previous example's weighted-sum matmuls between chunks.
                # Emitted at chunk TOP so later same-engine waits imply them.
                if pending is not None:
                    emit_mm4_slice(pending, ch * T_PER_CH, (ch + 1) * T_PER_CH)
                # load 1024 seq rows, cast fp32->bf16 during DMA (SWDGE --
                # the only casting path; gpsimd carries nothing else so
                # chunk 0's descriptors go out at t~0). Seq-tile relabeling:
                # tile tt = r holds rows s = p*8 + r, so each descriptor
                # covers 8 contiguous HBM rows (8KB source runs, one
                # descriptor per partition per chunk). Softmax/weighted-sum
                # are invariant to which rows form a tile as long as every
                # consumer derives from the same loaded layout.
                x_ch = xnat_pool.tile([128, T_PER_CH, C], BF16, tag="xnat")
                src = x_ext[b, ch * T_PER_CH * 128 : (ch + 1) * T_PER_CH * 128, :]
                rs = src.rearrange("(p r) c -> p r c", p=128, r=T_PER_CH)
                if bi == 0 and ch == 0:
                    # Split the pipeline-priming chunk into 4 sub-loads so the
                    # first transposes start after ~256KB instead of ~1MB.
                    # (A finer 128KB-first ladder measured ~2us WORSE -- the
                    # extra DMA's fixed cost exceeds its latency benefit.)
                    for rp in range(4):
                        nc.gpsimd.dma_start(
                            out=x_ch[:, 2 * rp : 2 * rp + 2, :],
                            in_=rs[:, 2 * rp : 2 * rp + 2, :],
                        )
                else:
                    nc.gpsimd.dma_start(out=x_ch[:], in_=rs)
                x_chunks.append(x_ch)

                # (7) TensorE transpose of each [128,128] block into bf16
                # PSUM, per c-half (one PSUM bank each); VectorE drains
                # PSUM -> SBUF (cheap: bf16 2x mode)
                ps_xt0 = psum_xt.tile([128, 1024], BF16, tag="ps_xt0")
                ps_xt1 = psum_xt.tile([128, 1024], BF16, tag="ps_xt1")
                for t in range(T_PER_CH):
                    nc.tensor.transpose(
                        ps_xt0[:, t * 128 : (t + 1) * 128],
                        x_ch[:, t, 0:128],
                        ident_sb[:],
                    )
                    nc.tensor.transpose(
                        ps_xt1[:, t * 128 : (t + 1) * 128],
                        x_ch[:, t, 128:256],
                        ident_sb[:],
                    )
                xt0 = xt_pool.tile([128, 1024], BF16, tag="xt0")
                xt1 = xt_pool.tile([128, 1024], BF16, tag="xt1")
                nc.vector.tensor_copy(xt0[:], ps_xt0[:])
                nc.vector.tensor_copy(xt1[:], ps_xt1[:])

                # Both mm1 pairs first, then both tanh+mm2 groups: tanh[s0]
                # (ScalarE) runs under mm1[s1]'s 512-col streams, and
                # tanh[s1] under mm2[s0], so the mm2 LDWs rarely wait.
                h_ts = []
                for sub in range(2):
                    s0 = sub * 512
                    hT_ps = psum_ht.tile([128, 512], F32, tag="hT")
                    nc.tensor.matmul(
                        hT_ps[:], w1_sb[:, 0, :], xt0[:, s0 : s0 + 512],
                        start=True, stop=False,
                    )
                    nc.tensor.matmul(
                        hT_ps[:], w1_sb[:, 1, :], xt1[:, s0 : s0 + 512],
                        start=False, stop=True,
                    )
                    h_t = ht_pool.tile([128, 512], BF16, tag="ht")
                    nc.scalar.activation(h_t[:], hT_ps[:], AF.Tanh, bias=b1_sb[:])
                    h_ts.append(h_t)
                for sub in range(2):
                    for j in range(4):
                        jj = ch * T_PER_CH + sub * 4 + j
                        nc.tensor.matmul(
                            scoresT_ps[:, jj : jj + 1],
                            h_ts[sub][:, j * 128 : (j + 1) * 128],
                            w2_sb[:],
                            start=True,
                            stop=True,
                        )

                if is_last:
                    # Tail weave: exp this chunk's just-finished score columns
                    # (no accum) and immediately emit its mm4s, so the last
                    # example's weighted sum overlaps its own score phase
                    # instead of serializing after it.
                    c0 = ch * T_PER_CH
                    nc.scalar.activation(
                        p_bf[:, c0 : c0 + T_PER_CH],
                        scoresT_ps[:, c0 : c0 + T_PER_CH],
                        AF.Exp,
                    )
                    emit_mm4_slice(last_st, c0, c0 + T_PER_CH)

                if pending is not None and ch == CH - 1:
                    finish_phase_b(pending)
                    pending = None

            # ---- this example's softmax reductions ----
            if is_last:
                # p_bf columns were produced per-chunk above; re-exp the full
                # scores into a throwaway tile just for the fused row-sum.
                p2_bf = small_pool.tile([128, S_TILES], BF16, tag="p2")
                sumrow = small_pool.tile([128, 1], F32, tag="sumrow")
                nc.scalar.activation(
                    p2_bf[:], scoresT_ps[:, 0:S_TILES], AF.Exp, accum_out=sumrow[:]
                )
            else:
                p_bf = small_pool.tile([128, S_TILES], BF16, tag="p")
                sumrow = small_pool.tile([128, 1], F32, tag="sumrow")
                nc.scalar.activation(
                    p_bf[:], scoresT_ps[:, 0:S_TILES], AF.Exp, accum_out=sumrow[:]
                )
            # cross-partition exp-sum lands in the scores tile's spare column
            # (same PSUM bank -> no extra slot, and the matmul's only wait is
            # the ScalarE accum above)
            nc.tensor.matmul(
                scoresT_ps[0:1, S_TILES : S_TILES + 1],
                ones_sb[:],
                sumrow[:],
                start=True,
                stop=True,
            )
            # give the DVE an up-to-date ScalarE observation so the
            # reciprocal's only explicit wait is the PE (sum matmul)
            dve_obs = small_pool.tile([1, 1], BF16, tag="dve_obs")
            nc.vector.tensor_copy(dve_obs[:], p_bf[0:1, 0:1] if not is_last else p2_bf[0:1, 0:1])
            recip = small_pool.tile([1, 1], F32, tag="recip")
            nc.vector.reciprocal(recip[:], scoresT_ps[0:1, S_TILES : S_TILES + 1])
            if is_last:
                last_st["recip"] = recip
                finish_phase_b(last_st)
            else:
                pending = {
                    "b": b,
                    "p_bf": p_bf,
                    "x_chunks": x_chunks,
                    "recip": recip,
                    "acc": acc_ps,
                }

    if strip_waits:
        _strip_implied_self_waits(nc)
    return nc


def _strip_implied_self_waits(nc: bass.Bass) -> None:
    """Reduce per-instruction sync waits to what the hardware needs.

    Walrus accepts at most ONE sync wait per engine ISA instruction, but
    Tile emits waits per logical dependency. Two sound reductions:

    1. Engine-clock elision. Each engine's sequencer evaluates waits in
       program order and engines retire in order, so if an earlier
       instruction on the SAME engine already waited for sem >= v' (v'>=v),
       a later instruction's wait for sem >= v is vacuous: the semaphore
       condition held before the predecessor issued. (Tile deliberately
       doesn't do this transitive per-proc minimization.) Also covers waits
       on the engine's own completion semaphore.

    2. x-load WAW elision. Each x-chunk load carries {PE >= k (WAR: all
       readers of the slot's old contents are done), DMASW >= v (WAW vs the
       old writer)}. The readers read-after-wrote the old data, so the WAR
       wait transitively dominates the WAW wait; drop the DMASW wait.
    """
    eng_prefix = {
        mybir.EngineType.PE: "PE_",
        mybir.EngineType.DVE: "DVE_",
        mybir.EngineType.Activation: "Activation_",
        mybir.EngineType.Pool: "Pool_",
        mybir.EngineType.SP: "SP_",
    }
    # Sems that are ever non-monotonically updated (barrier gather/release
    # use sem-sub) are excluded from all reasoning: their values regress.
    nonmono: set[str] = set()
    for f in nc.m.functions:
        for blk in f.blocks:
            for inst in blk.instructions:
                si = inst.sync_info
                if si is None:
                    continue
                for u in si.on_update:
                    if u.sync_type == "semaphore" and u.update_mode not in (
                        "sem-inc",
                        "sem-add-imm",
                    ):
                        nonmono.add(u.ant_name)

    observed: dict[mybir.EngineType, dict[str, int]] = {}
    for f in nc.m.functions:
        for blk in f.blocks:
            splits: list[tuple[int, list]] = []
            for idx, inst in enumerate(blk.instructions):
                si = inst.sync_info
                if si is None:
                    continue
                tn = type(inst).__name__
                if tn == "InstEventSemaphore":
                    continue  # barrier machinery: leave untouched
                eng = inst.engine
                obs = observed.setdefault(eng, {})
                pref = eng_prefix.get(eng)
                is_x_load = False
                if tn == "InstDMACopy" and eng == mybir.EngineType.Pool:
                    try:
                        is_x_load = "x_ch" in str(inst.outs[0])
                    except Exception:
                        is_x_load = False
                has_pe_wait = any(
                    w.sync_type == "semaphore" and w.ant_name.startswith("PE_")
                    for w in si.on_wait
                )
                kept = []
                for w in si.on_wait:
                    if (
                        w.sync_type != "semaphore"
                        or w.wait_mode != "sem-ge-imm"
                        or w.ant_name in nonmono
                        or tn == "InstDrain"
                    ):
                        kept.append(w)
                        continue
                    # (1) engine-clock / self-wait elision
                    if obs.get(w.ant_name, 0) >= w.wait_value:
                        continue
                    # (2) x-load WAW-vs-old-writer elision
                    if (
                        is_x_load
                        and has_pe_wait
                        and w.ant_name.startswith("DMASW")
                    ):
                        continue
                    kept.append(w)
                # record knowledge from ALL original waits (sound even for
                # stripped ones: the condition held at this program point)
                for w in si.on_wait:
                    if (
                        w.sync_type == "semaphore"
                        and w.wait_mode == "sem-ge-imm"
                        and w.ant_name not in nonmono
                    ):
                        if obs.get(w.ant_name, 0) < w.wait_value:
                            obs[w.ant_name] = w.wait_value
                if len(kept) != len(si.on_wait):
                    si.on_wait = kept
                    kept = si.on_wait  # re-read normalized
                if len(kept) > 1:
                    # Hardware takes one sync wait per instruction: carry the
                    # surplus on single-wait Drain instructions inserted just
                    # before (same engine => sequencer evaluates them first).
                    extras = []
                    for i, w in enumerate(kept[:-1]):
                        d = mybir.InstDrain(
                            name=f"{inst.name}-w{i}", ins=[], outs=[]
                        )
                        d.engine = inst.engine
                        d.sync_info = mybir.SyncInfo(on_wait=[w], on_update=[])
                        extras.append(d)
                    si.on_wait = [kept[-1]]
                    splits.append((idx, extras))
                # engine-own completion increments advance the engine clock.
                # Pool excluded: its 8 Q7 cores may retire out of order, so
                # completion-count knowledge is only valid for strict-FIFO
                # engines (wait-observation inheritance above is still valid
                # for Pool -- the NX sequencer evaluates waits in order).
                if pref is not None and eng != mybir.EngineType.Pool:
                    for u in si.on_update:
                        if (
                            u.sync_type == "semaphore"
                            and u.update_mode in ("sem-inc", "sem-add-imm")
                            and u.ant_name.startswith(pref)
                        ):
                            obs[u.ant_name] = obs.get(u.ant_name, 0) + (
                                u.update_value or 1
                            )
            if splits:
                il = blk.instructions
                for idx, extras in reversed(splits):
                    for d in reversed(extras):
                        il.insert(idx, d)


_NC_CACHE = None


def _get_nc() -> bass.Bass:
    global _NC_CACHE
    if _NC_CACHE is None:
        _NC_CACHE = build_nc()
    return _NC_CACHE


def kernel(x, W1, b1, w2, b2=None, **_unused) -> np.ndarray:
    """Full-input entry point: shard batch across 8 cores, run, gather.

    b2 is mathematically irrelevant (softmax shift invariance) and ignored.
    """
    x = np.ascontiguousarray(np.asarray(x, dtype=np.float32))
    W1 = np.ascontiguousarray(np.asarray(W1, dtype=np.float32))
    b1 = np.ascontiguousarray(np.asarray(b1, dtype=np.float32))
    w2 = np.ascontiguousarray(np.asarray(w2, dtype=np.float32))
    assert x.shape == (B, S, C), x.shape

    import ml_dtypes

    ident = np.eye(128, dtype=ml_dtypes.bfloat16)
    W1_bf = W1.astype(ml_dtypes.bfloat16)
    w2_bf = w2.astype(ml_dtypes.bfloat16)
    nc = _get_nc()
    in_maps = [
        {
            "x": x[i * B_LOC : (i + 1) * B_LOC],
            "W1": W1_bf,
            "b1": b1,
            "w2": w2_bf,
            "ident": ident,
        }
        for i in range(N_CORES)
    ]
    res = run_bass_kernel_spmd(nc, in_maps, list(range(N_CORES))).results
    out = np.concatenate([res[i]["out"] for i in range(N_CORES)], axis=0)
    return out.astype(np.float32)



# revision 16
# speedup vs baseline: 1.0351x; 1.0351x over previous
"""AttentivePooling Trainium2 kernel.

Computes, per example b:
    h      = tanh(x[b] @ W1 + b1)          # (S, MID)
    scores = h @ w2 (+ b2, dropped: softmax-invariant)
    attn   = softmax(scores)               # over S
    out[b] = attn @ x[b]                   # (C,)

Sharding: batch (32) split across 8 NeuronCores -> 4 examples/core.
Weights replicated. No cross-core communication.

Per-core dataflow (on-chip tensors bf16, accumulation fp32):
  - x loaded HBM->SBUF once in 1MB chunks, fp32->bf16 cast during DMA
    (SWDGE; the only casting path), layout [s=128 partitions, (tile, c)
    free] with seq tiles relabeled so every descriptor covers 4
    contiguous HBM rows. Constants ride the Sync-engine HWDGE ring (W1/
    w2 host-pre-cast to bf16 since HWDGE cannot cast) so gpsimd emits
    x-chunk descriptors from t~0; the priming chunk is split into 4
    sub-loads so the first transposes start after ~256KB. Chunk size is
    a measured optimum: 512KB chunks cost +27us (per-DMA overheads),
    2MB chunks cost +5us and jitter.
  - A burst of 16 dummy N=512 matmuls on memset constants at t~0
    (while the first x chunk is in flight) trips the PE HAM clock gate
    out of its cold 1.2 GHz state before real matmuls arrive.
  - xT via TensorE transpose-mode matmuls ([128,128] bf16 blocks,
    identity moving) into bf16 PSUM tiles, drained to SBUF by VectorE
    copies. (The DMA xbar transpose path is unusable here: Tile globally
    serializes xbar transposes against normal DMAs, and the XPOSE ISA
    slot only carries one semaphore wait -- walrus rejects the 2-3 waits
    a pipelined kernel needs. fp32 pair-view transposes halve the
    instruction count but stream at 2 cycles/row and lose FWL on the
    weight load -- measured a wash, with fp32-mode pipeline pollution on
    neighboring matmuls. fp8 for the scores path fails the 2e-2 gate:
    rel err 2.3e-2 measured.)
  - mm1: hT[m, s] = W1k0.T @ xT[...,half0] + W1k1.T @ xT[...,half1] (PE).
    (Reordering to k0,k0,k1,k1 across two PSUM banks so consecutive
    matmuls share a stationary measured neutral -- the backend does NOT
    dedupe repeat LDWEIGHTS.)
  - tanh(. + b1) via ScalarE per-partition bias, bf16 out.
  - mm2: scoresT[:, jj] = h_chunk.T @ w2 -> scores in [128, 64] layout
    (seq position on partitions, seq tile as column) feeding both the
    softmax and mm4's stationary operand with no reshape.
  - softmax WITHOUT max subtraction: |scores| <= ||w2||_1 + |b2| ~ 5.7,
    exp is safe in fp32. exp via ScalarE with fused row-sum accum_out;
    cross-partition total via a ones-vector matmul; normalization
    deferred to the end (scale by 1/sum).
  - mm4: acc[strip t%4] += p[:, t].T @ x_tile over 64 seq tiles, with
    tile_position=(0, 32*(t%4)) col-group tiling: four M=1 matmuls with
    independent stationaries stream CONCURRENTLY through disjoint 32-col
    array strips (the per-tile LDWEIGHTS+stream no longer serializes;
    measured mm4 26.5us -> 15.2us). The four strip rows live at
    partitions {0,32,64,96} of a DEDICATED PSUM bank in ONE accumulation
    chain -- start=True clears has_written bank-wide, so per-strip
    chains would destroy each other; first-touch of rows 32/64/96 with
    start=False overwrites by has_written semantics (DVE memsets keep
    CoreSim's model in agreement). DVE sums the strips at example
    finish. Weighted sums are interleaved into the NEXT example's score
    phase so the in-order PE never stalls at example boundaries; the
    LAST example instead weaves its mm4s into its OWN chunks via
    per-chunk exp of the just-finished score columns (kills the
    end-of-kernel serial tail).

Measured: 133.8us (previous session) -> ~122.6us (this session; 122.5/
122.7 back-to-back in a healthy device window), rel err 2.19e-3,
DMA-roofline-dominated (HBM read of fp32 x: ~94us min; DMA engines
active ~104us; Tensor busy ~103us). The chip intermittently enters a
~15%-slower sustained-load power state (~145us) that affects any build
equally.
"""

from contextlib import ExitStack

import numpy as np

import concourse.bass as bass
import concourse.tile as tile
from concourse import mybir
from concourse.bass_utils import run_bass_kernel_spmd

B, S, C, MID = 32, 8192, 256, 128
N_CORES = 8
B_LOC = B // N_CORES  # 4 examples per core

F32 = mybir.dt.float32
BF16 = mybir.dt.bfloat16
AF = mybir.ActivationFunctionType

S_TILES = S // 128  # 64 seq tiles of 128 per example
CH = 8  # chunks per example (1024 seq each; measured optimum -- both
# halving (CH=16) and doubling (CH=4) the DMA size cost 3-27us)
T_PER_CH = S_TILES // CH  # 8 seq tiles per chunk


def build_nc(reps: int = 1, strip_waits: bool = True) -> bass.Bass:
    """Build the per-core program. reps>1 repeats the whole computation
    back-to-back inside one NEFF (used only for benchmarking: the wall-time
    difference between reps=R and reps=1 isolates kernel time from dispatch
    overhead). strip_waits=False keeps Tile's full (redundant) semaphore
    waits -- required for CoreSim, whose race detector doesn't model
    engine-FIFO-implied ordering; hardware builds need the strip because
    walrus allows only one sync wait per engine ISA instruction."""
    nc = bass.Bass("TRN2", target_bir_lowering=False, debug=False)

    x_ext = nc.declare_dram_parameter("x", [B_LOC, S, C], F32, isOutput=False)
    # W1/w2 arrive host-pre-cast to bf16 (tiny constants) so their loads can
    # ride the Sync HWDGE ring -- only gpsimd DMAs can cast, and gpsimd must
    # stay free for the x-chunk pipeline from t=0.
    w1_ext = nc.declare_dram_parameter("W1", [C, MID], BF16, isOutput=False)
    b1_ext = nc.declare_dram_parameter("b1", [MID], F32, isOutput=False)
    w2_ext = nc.declare_dram_parameter("w2", [MID], BF16, isOutput=False)
    id_ext = nc.declare_dram_parameter("ident", [128, 128], BF16, isOutput=False)
    out_ext = nc.declare_dram_parameter("out", [B_LOC, C], F32, isOutput=True)

    with tile.TileContext(nc) as tc, ExitStack() as ctx:
        const_pool = ctx.enter_context(tc.tile_pool(name="const", bufs=1))
        # All 32 chunks resident (128KB/partition): no slot reuse means no
        # WAW pacing waits in the DMA queues -- the whole x load streams
        # back-to-back, so the E64 engine (which also serves the ~8us of
        # instruction-fetch DMAs) drains its share continuously instead of
        # crawling through a lone-engine backlog at the end of the load.
        xnat_pool = ctx.enter_context(tc.tile_pool(name="xnat", bufs=4 * CH))
        xt_pool = ctx.enter_context(tc.tile_pool(name="xt", bufs=3))
        ht_pool = ctx.enter_context(tc.tile_pool(name="ht", bufs=4))
        small_pool = ctx.enter_context(tc.tile_pool(name="small", bufs=4))
        psum_xt = ctx.enter_context(tc.tile_pool(name="psum_xt", bufs=1, space="PSUM"))
        psum_ht = ctx.enter_context(tc.tile_pool(name="psum_ht", bufs=2, space="PSUM"))
        psum_sc = ctx.enter_context(tc.tile_pool(name="psum_sc", bufs=2, space="PSUM"))
        psum_oa = ctx.enter_context(tc.tile_pool(name="psum_oa", bufs=2, space="PSUM"))

        # ---- HAM warm-up burst (no DMA dependencies: memset-fed) ----
        ones_bf = const_pool.tile([128, 1], BF16, tag="ones_bf")
        nc.vector.memset(ones_bf[:], 1.0)
        ones_sb = const_pool.tile([128, 1], F32, tag="ones")
        nc.vector.memset(ones_sb[:], 1.0)
        wburst_sb = const_pool.tile([128, 512], BF16, tag="wburst")
        nc.vector.memset(wburst_sb[:], 1.0)
        warm_ps = psum_ht.tile([128, 512], F32, tag="hT")
        for _ in range(16):
            nc.tensor.matmul(
                warm_ps[0:1, :], ones_bf[:], wburst_sb[:], start=True, stop=True
            )

        # ---- constants (one-time, tiny; Sync-engine HWDGE ring so they
        # don't queue behind x-chunk SWDGE descriptor generation) ----
        w1_sb = const_pool.tile([128, 2, MID], BF16, tag="w1")
        nc.sync.dma_start(
            out=w1_sb[:], in_=w1_ext[:].rearrange("(k p) m -> p k m", p=128)
        )
        w2_sb = const_pool.tile([128, 1], BF16, tag="w2")
        nc.sync.dma_start(out=w2_sb[:], in_=w2_ext[:].rearrange("(p o) -> p o", o=1))
        b1_sb = const_pool.tile([128, 1], F32, tag="b1")
        nc.sync.dma_start(out=b1_sb[:], in_=b1_ext[:].rearrange("(p o) -> p o", o=1))
        ident_sb = const_pool.tile([128, 128], BF16, tag="ident")
        nc.sync.dma_start(out=ident_sb[:], in_=id_ext[:])
        # Warm up the ScalarE activation table (exp_and_others: tanh+exp) on a
        # tiny input. Walrus attaches the table-load sync to the first
        # ACTIVATE; without this, that instruction exceeds the ISA's
        # 2-sync-wait budget once Tile's own deps are added.
        warm_sb = const_pool.tile([128, 1], F32, tag="warm")
        nc.scalar.activation(warm_sb[:], b1_sb[:], AF.Tanh, bias=b1_sb[:])
        # Warm the other engines too, and -- crucially -- make the PE observe
        # every constant's DMA-lane semaphore via one-wait warmup matmuls, so
        # no steady-state matmul ever needs a second (constant-load) wait.
        # Walrus enforces at most ONE sync wait per engine ISA instruction.
        warm2_sb = const_pool.tile([128, 1], F32, tag="warm2")
        nc.vector.tensor_copy(warm2_sb[:], ones_sb[:])
        nc.tensor.matmul(
            warm_ps[0:1, 0:1], ones_sb[:], ones_sb[:], start=True, stop=True
        )
        nc.tensor.matmul(
            warm_ps[0:1, 0:1], ident_sb[:, 0:1], ones_bf[:], start=True, stop=True
        )
        nc.tensor.matmul(
            warm_ps[0:1, 0:1], w1_sb[:, 0, 0:1], ones_bf[:], start=True, stop=True
        )
        nc.tensor.matmul(
            warm_ps[0:1, 0:1], ones_bf[:], w2_sb[:], start=True, stop=True
        )

        def emit_mm4_slice(st, lo, hi):
            """Accumulate seq tiles [lo, hi) of a pending example into the
            4 col-group strip accumulators (partitions 0/32/64/96)."""
            acc = st["acc"]
            for t in range(lo, hi):
                j = t % 4
                # skip_group_check: CoreSim's zero-region tracker flattens
                # base_partition!=0 PSUM APs to bogus (partition, bank) keys,
                # spuriously colliding with other pools. The strips write
                # disjoint partitions {0,32,64,96} of one bank; per-element
                # has_written accumulation is genuinely race-free.
                nc.tensor.matmul(
                    acc[32 * j : 32 * j + 1, :],
                    st["p_bf"][:, t : t + 1],
                    st["x_chunks"][t // T_PER_CH][:, t % T_PER_CH, :],
                    start=(t == 0),
                    stop=(t == S_TILES - 1),
                    tile_position=(0, 32 * j),
                    skip_group_check=True,
                )

        def finish_phase_b(st):
            # DVE reads at most ONE non-scalar operand from PSUM per
            # instruction (NCC_IBVF027): chain the strip adds through SBUF.
            acc = st["acc"]
            t0 = small_pool.tile([1, C], F32, tag="t0")
            nc.vector.tensor_copy(t0[:], acc[0:1, :])
            t1 = small_pool.tile([1, C], F32, tag="t1")
            nc.vector.tensor_tensor(t1[:], t0[:], acc[32:33, :], mybir.AluOpType.add)
            t2 = small_pool.tile([1, C], F32, tag="t2")
            nc.vector.tensor_tensor(t2[:], t1[:], acc[64:65, :], mybir.AluOpType.add)
            tsum = small_pool.tile([1, C], F32, tag="tsum")
            nc.vector.tensor_tensor(
                tsum[:], t2[:], acc[96:97, :], mybir.AluOpType.add
            )
            out_sb = small_pool.tile([1, C], F32, tag="out_sb")
            nc.vector.tensor_scalar_mul(out_sb[:], tsum[:], st["recip"][:])
            nc.scalar.dma_start(out=out_ext[st["b"] : st["b"] + 1, :], in_=out_sb[:])

        pending = None  # example whose weighted-sum mm4s are being woven
        blist = [bb for _ in range(reps) for bb in range(B_LOC)]
        for bi, b in enumerate(blist):
            is_last = bi == len(blist) - 1
            x_chunks = []
            # scoresT cols [0:64]; col [64] holds the cross-partition exp-sum.
            scoresT_ps = psum_sc.tile([128, S_TILES + 1], F32, tag="scoresT")
            # Dedicated accumulator bank: 4 col-group strip rows at partitions
            # {0,32,64,96}, ONE accumulation chain (start only at t=0 -- any
            # start=True clears has_written for the WHOLE bank, so per-strip
            # chains in a shared bank would destroy each other). Rows 32/64/96
            # are first written with start=False (has_written clear -> HW
            # overwrites); the DVE memsets below make CoreSim's accumulate-
            # based model agree with that overwrite semantics.
            acc_ps = psum_oa.tile([128, C], F32, tag="outacc")
            for j in (32, 64, 96):
                nc.vector.memset(acc_ps[j : j + 1, :], 0.0)
            if is_last:
                p_bf = small_pool.tile([128, S_TILES], BF16, tag="p")
                last_st = {"acc": acc_ps, "x_chunks": x_chunks, "b": b,
                           "p_bf": p_bf}
            for ch in range(CH):
                # keep the PE busy across example boundaries: weave the
                # previous example's weighted-sum matmuls between chunks.
                # Emitted at chunk TOP so later same-engine waits imply them.
                if pending is not None:
                    emit_mm4_slice(pending, ch * T_PER_CH, (ch + 1) * T_PER_CH)
                # load 1024 seq rows, cast fp32->bf16 during DMA (SWDGE --
                # the only casting path; gpsimd carries nothing else so
                # chunk 0's descriptors go out at t~0). Seq-tile relabeling:
                # tile tt = r holds rows s = p*8 + r, so each descriptor
                # covers 8 contiguous HBM rows (8KB source runs, one
                # descriptor per partition per chunk). Softmax/weighted-sum
                # are invariant to which rows form a tile as long as every
                # consumer derives from the same loaded layout.
                x_ch = xnat_pool.tile([128, T_PER_CH, C], BF16, tag="xnat")
                src = x_ext[b, ch * T_PER_CH * 128 : (ch + 1) * T_PER_CH * 128, :]
                rs = src.rearrange("(p r) c -> p r c", p=128, r=T_PER_CH)
                if bi == 0 and ch == 0:
                    # Split the pipeline-priming chunk [1,1,2,4] so the first
                    # transposes start after ~128KB: the DMA engines run COLD
                    # (~130 GB/s) until the HAM ramp at ~11us, so the first
                    # sub-load's size directly sets when the PE starts.
                    for lo, hi in ((0, 1), (1, 2), (2, 4), (4, 8)):
                        nc.gpsimd.dma_start(
                            out=x_ch[:, lo:hi, :],
                            in_=rs[:, lo:hi, :],
                        )
                else:
                    nc.gpsimd.dma_start(out=x_ch[:], in_=rs)
                x_chunks.append(x_ch)

                # (7) TensorE transpose of each [128,128] block into bf16
                # PSUM, per c-half (one PSUM bank each); VectorE drains
                # PSUM -> SBUF (cheap: bf16 2x mode)
                ps_xt0 = psum_xt.tile([128, 1024], BF16, tag="ps_xt0")
                ps_xt1 = psum_xt.tile([128, 1024], BF16, tag="ps_xt1")
                for t in range(T_PER_CH):
                    nc.tensor.transpose(
                        ps_xt0[:, t * 128 : (t + 1) * 128],
                        x_ch[:, t, 0:128],
                        ident_sb[:],
                    )
                    nc.tensor.transpose(
                        ps_xt1[:, t * 128 : (t + 1) * 128],
                        x_ch[:, t, 128:256],
                        ident_sb[:],
                    )
                xt0 = xt_pool.tile([128, 1024], BF16, tag="xt0")
                xt1 = xt_pool.tile([128, 1024], BF16, tag="xt1")
                nc.vector.tensor_copy(xt0[:], ps_xt0[:])
                nc.vector.tensor_copy(xt1[:], ps_xt1[:])

                # Both mm1 pairs first, then both tanh+mm2 groups: tanh[s0]
                # (ScalarE) runs under mm1[s1]'s 512-col streams, and
                # tanh[s1] under mm2[s0], so the mm2 LDWs rarely wait.
                h_ts = []
                for sub in range(2):
                    s0 = sub * 512
                    hT_ps = psum_ht.tile([128, 512], F32, tag="hT")
                    nc.tensor.matmul(
                        hT_ps[:], w1_sb[:, 0, :], xt0[:, s0 : s0 + 512],
                        start=True, stop=False,
                    )
                    nc.tensor.matmul(
                        hT_ps[:], w1_sb[:, 1, :], xt1[:, s0 : s0 + 512],
                        start=False, stop=True,
                    )
                    h_t = ht_pool.tile([128, 512], BF16, tag="ht")
                    nc.scalar.activation(h_t[:], hT_ps[:], AF.Tanh, bias=b1_sb[:])
                    h_ts.append(h_t)
                for sub in range(2):
                    for j in range(4):
                        jj = ch * T_PER_CH + sub * 4 + j
                        nc.tensor.matmul(
                            scoresT_ps[:, jj : jj + 1],
                            h_ts[sub][:, j * 128 : (j + 1) * 128],
                            w2_sb[:],
                            start=True,
                            stop=True,
                        )

                if is_last:
                    # Tail weave: exp this chunk's just-finished score columns
                    # (no accum) and immediately emit its mm4s, so the last
                    # example's weighted sum overlaps its own score phase
                    # instead of serializing after it.
                    c0 = ch * T_PER_CH
                    nc.scalar.activation(
                        p_bf[:, c0 : c0 + T_PER_CH],
                        scoresT_ps[:, c0 : c0 + T_PER_CH],
                        AF.Exp,
                    )
                    emit_mm4_slice(last_st, c0, c0 + T_PER_CH)

                if pending is not None and ch == CH - 1:
                    finish_phase_b(pending)
                    pending = None

            # ---- this example's softmax reductions ----
            if is_last:
                # p_bf columns were produced per-chunk above; re-exp the full
                # scores into a throwaway tile just for the fused row-sum.
                p2_bf = small_pool.tile([128, S_TILES], BF16, tag="p2")
                sumrow = small_pool.tile([128, 1], F32, tag="sumrow")
                nc.scalar.activation(
                    p2_bf[:], scoresT_ps[:, 0:S_TILES], AF.Exp, accum_out=sumrow[:]
                )
            else:
                p_bf = small_pool.tile([128, S_TILES], BF16, tag="p")
                sumrow = small_pool.tile([128, 1], F32, tag="sumrow")
                nc.scalar.activation(
                    p_bf[:], scoresT_ps[:, 0:S_TILES], AF.Exp, accum_out=sumrow[:]
                )
            # cross-partition exp-sum lands in the scores tile's spare column
            # (same PSUM bank -> no extra slot, and the matmul's only wait is
            # the ScalarE accum above)
            nc.tensor.matmul(
                scoresT_ps[0:1, S_TILES : S_TILES + 1],
                ones_sb[:],
                sumrow[:],
                start=True,
                stop=True,
            )
            # give the DVE an up-to-date ScalarE observation so the
            # reciprocal's only explicit wait is the PE (sum matmul)
            dve_obs = small_pool.tile([1, 1], BF16, tag="dve_obs")
            nc.vector.tensor_copy(dve_obs[:], p_bf[0:1, 0:1] if not is_last else p2_bf[0:1, 0:1])
            recip = small_pool.tile([1, 1], F32, tag="recip")
            nc.vector.reciprocal(recip[:], scoresT_ps[0:1, S_TILES : S_TILES + 1])
            if is_last:
                last_st["recip"] = recip
                finish_phase_b(last_st)
            else:
                pending = {
                    "b": b,
                    "p_bf": p_bf,
                    "x_chunks": x_chunks,
                    "recip": recip,
                    "acc": acc_ps,
                }

    if strip_waits:
        _strip_implied_self_waits(nc)
    return nc


def _strip_implied_self_waits(nc: bass.Bass) -> None:
    """Reduce per-instruction sync waits to what the hardware needs.

    Walrus accepts at most ONE sync wait per engine ISA instruction, but
    Tile emits waits per logical dependency. Two sound reductions:

    1. Engine-clock elision. Each engine's sequencer evaluates waits in
       program order and engines retire in order, so if an earlier
       instruction on the SAME engine already waited for sem >= v' (v'>=v),
       a later instruction's wait for sem >= v is vacuous: the semaphore
       condition held before the predecessor issued. (Tile deliberately
       doesn't do this transitive per-proc minimization.) Also covers waits
       on the engine's own completion semaphore.

    2. x-load WAW elision. Each x-chunk load carries {PE >= k (WAR: all
       readers of the slot's old contents are done), DMASW >= v (WAW vs the
       old writer)}. The readers read-after-wrote the old data, so the WAR
       wait transitively dominates the WAW wait; drop the DMASW wait.
    """
    eng_prefix = {
        mybir.EngineType.PE: "PE_",
        mybir.EngineType.DVE: "DVE_",
        mybir.EngineType.Activation: "Activation_",
        mybir.EngineType.Pool: "Pool_",
        mybir.EngineType.SP: "SP_",
    }
    # Sems that are ever non-monotonically updated (barrier gather/release
    # use sem-sub) are excluded from all reasoning: their values regress.
    nonmono: set[str] = set()
    for f in nc.m.functions:
        for blk in f.blocks:
            for inst in blk.instructions:
                si = inst.sync_info
                if si is None:
                    continue
                for u in si.on_update:
                    if u.sync_type == "semaphore" and u.update_mode not in (
                        "sem-inc",
                        "sem-add-imm",
                    ):
                        nonmono.add(u.ant_name)

    observed: dict[mybir.EngineType, dict[str, int]] = {}
    for f in nc.m.functions:
        for blk in f.blocks:
            splits: list[tuple[int, list]] = []
            for idx, inst in enumerate(blk.instructions):
                si = inst.sync_info
                if si is None:
                    continue
                tn = type(inst).__name__
                if tn == "InstEventSemaphore":
                    continue  # barrier machinery: leave untouched
                eng = inst.engine
                obs = observed.setdefault(eng, {})
                pref = eng_prefix.get(eng)
                is_x_load = False
                if tn == "InstDMACopy" and eng == mybir.EngineType.Pool:
                    try:
                        is_x_load = "x_ch" in str(inst.outs[0])
                    except Exception:
                        is_x_load = False
                has_pe_wait = any(
                    w.sync_type == "semaphore" and w.ant_name.startswith("PE_")
                    for w in si.on_wait
                )
                kept = []
                for w in si.on_wait:
                    if (
                        w.sync_type != "semaphore"
                        or w.wait_mode != "sem-ge-imm"
                        or w.ant_name in nonmono
                        or tn == "InstDrain"
                    ):
                        kept.append(w)
                        continue
                    # (1) engine-clock / self-wait elision
                    if obs.get(w.ant_name, 0) >= w.wait_value:
                        continue
                    # (2) x-load WAW-vs-old-writer elision
                    if (
                        is_x_load
                        and has_pe_wait
                        and w.ant_name.startswith("DMASW")
                    ):
                        continue
                    kept.append(w)
                # record knowledge from ALL original waits (sound even for
                # stripped ones: the condition held at this program point)
                for w in si.on_wait:
                    if (
                        w.sync_type == "semaphore"
                        and w.wait_mode == "sem-ge-imm"
                        and w.ant_name not in nonmono
                    ):
                        if obs.get(w.ant_name, 0) < w.wait_value:
                            obs[w.ant_name] = w.wait_value
                if len(kept) != len(si.on_wait):
                    si.on_wait = kept
                    kept = si.on_wait  # re-read normalized
                if len(kept) > 1:
                    # Hardware takes one sync wait per instruction: carry the
                    # surplus on single-wait Drain instructions inserted just
                    # before (same engine => sequencer evaluates them first).
                    extras = []
                    for i, w in enumerate(kept[:-1]):
                        d = mybir.InstDrain(
                            name=f"{inst.name}-w{i}", ins=[], outs=[]
                        )
                        d.engine = inst.engine
                        d.sync_info = mybir.SyncInfo(on_wait=[w], on_update=[])
                        extras.append(d)
                    si.on_wait = [kept[-1]]
                    splits.append((idx, extras))
                # engine-own completion increments advance the engine clock.
                # Pool excluded: its 8 Q7 cores may retire out of order, so
                # completion-count knowledge is only valid for strict-FIFO
                # engines (wait-observation inheritance above is still valid
                # for Pool -- the NX sequencer evaluates waits in order).
                if pref is not None and eng != mybir.EngineType.Pool:
                    for u in si.on_update:
                        if (
                            u.sync_type == "semaphore"
                            and u.update_mode in ("sem-inc", "sem-add-imm")
                            and u.ant_name.startswith(pref)
                        ):
                            obs[u.ant_name] = obs.get(u.ant_name, 0) + (
                                u.update_value or 1
                            )
            if splits:
                il = blk.instructions
                for idx, extras in reversed(splits):
                    for d in reversed(extras):
                        il.insert(idx, d)


_NC_CACHE = None


def _get_nc() -> bass.Bass:
    global _NC_CACHE
    if _NC_CACHE is None:
        _NC_CACHE = build_nc()
    return _NC_CACHE


def kernel(x, W1, b1, w2, b2=None, **_unused) -> np.ndarray:
    """Full-input entry point: shard batch across 8 cores, run, gather.

    b2 is mathematically irrelevant (softmax shift invariance) and ignored.
    """
    x = np.ascontiguousarray(np.asarray(x, dtype=np.float32))
    W1 = np.ascontiguousarray(np.asarray(W1, dtype=np.float32))
    b1 = np.ascontiguousarray(np.asarray(b1, dtype=np.float32))
    w2 = np.ascontiguousarray(np.asarray(w2, dtype=np.float32))
    assert x.shape == (B, S, C), x.shape

    import ml_dtypes

    ident = np.eye(128, dtype=ml_dtypes.bfloat16)
    W1_bf = W1.astype(ml_dtypes.bfloat16)
    w2_bf = w2.astype(ml_dtypes.bfloat16)
    nc = _get_nc()
    in_maps = [
        {
            "x": x[i * B_LOC : (i + 1) * B_LOC],
            "W1": W1_bf,
            "b1": b1,
            "w2": w2_bf,
            "ident": ident,
        }
        for i in range(N_CORES)
    ]
    res = run_bass_kernel_spmd(nc, in_maps, list(range(N_CORES))).results
    out = np.concatenate([res[i]["out"] for i in range(N_CORES)], axis=0)
    return out.astype(np.float32)



# revision 20
# speedup vs baseline: 1.0756x; 1.0391x over previous
"""AttentivePooling Trainium2 kernel.

Computes, per example b:
    h      = tanh(x[b] @ W1 + b1)          # (S, MID)
    scores = h @ w2 (+ b2, dropped: softmax-invariant)
    attn   = softmax(scores)               # over S
    out[b] = attn @ x[b]                   # (C,)

Sharding: batch (32) split across 8 NeuronCores -> 4 examples/core.
Weights replicated. No cross-core communication.

Per-core dataflow (on-chip tensors bf16, accumulation fp32):
  - x loaded HBM->SBUF once in 1MB chunks, fp32->bf16 cast during DMA
    (SWDGE; the only casting path), layout [s=128 partitions, (tile, c)
    free] with seq tiles relabeled so every descriptor covers 4
    contiguous HBM rows. Constants ride the Sync-engine HWDGE ring (W1/
    w2 host-pre-cast to bf16 since HWDGE cannot cast) so gpsimd emits
    x-chunk descriptors from t~0; the priming chunk is split into 4
    sub-loads so the first transposes start after ~256KB. Chunk size is
    a measured optimum: 512KB chunks cost +27us (per-DMA overheads),
    2MB chunks cost +5us and jitter.
  - A burst of 16 dummy N=512 matmuls on memset constants at t~0
    (while the first x chunk is in flight) trips the PE HAM clock gate
    out of its cold 1.2 GHz state before real matmuls arrive.
  - xT via TensorE transpose-mode matmuls ([128,128] bf16 blocks,
    identity moving) into bf16 PSUM tiles, drained to SBUF by VectorE
    copies. (The DMA xbar transpose path is unusable here: Tile globally
    serializes xbar transposes against normal DMAs, and the XPOSE ISA
    slot only carries one semaphore wait -- walrus rejects the 2-3 waits
    a pipelined kernel needs. fp32 pair-view transposes halve the
    instruction count but stream at 2 cycles/row and lose FWL on the
    weight load -- measured a wash, with fp32-mode pipeline pollution on
    neighboring matmuls. fp8 for the scores path fails the 2e-2 gate:
    rel err 2.3e-2 measured.)
  - mm1: hT[m, s] = W1k0.T @ xT[...,half0] + W1k1.T @ xT[...,half1] (PE).
    (Reordering to k0,k0,k1,k1 across two PSUM banks so consecutive
    matmuls share a stationary measured neutral -- the backend does NOT
    dedupe repeat LDWEIGHTS.)
  - tanh(. + b1) via ScalarE per-partition bias, bf16 out.
  - mm2: scoresT[:, jj] = h_chunk.T @ w2 -> scores in [128, 64] layout
    (seq position on partitions, seq tile as column) feeding both the
    softmax and mm4's stationary operand with no reshape.
  - softmax WITHOUT max subtraction: |scores| <= ||w2||_1 + |b2| ~ 5.7,
    exp is safe in fp32. exp via ScalarE with fused row-sum accum_out;
    cross-partition total via a ones-vector matmul; normalization
    deferred to the end (scale by 1/sum).
  - mm4: acc[strip t%4] += p[:, t].T @ x_tile over 64 seq tiles, with
    tile_position=(0, 32*(t%4)) col-group tiling: four M=1 matmuls with
    independent stationaries stream CONCURRENTLY through disjoint 32-col
    array strips (the per-tile LDWEIGHTS+stream no longer serializes;
    measured mm4 26.5us -> 15.2us). The four strip rows live at
    partitions {0,32,64,96} of a DEDICATED PSUM bank in ONE accumulation
    chain -- start=True clears has_written bank-wide, so per-strip
    chains would destroy each other; first-touch of rows 32/64/96 with
    start=False overwrites by has_written semantics (DVE memsets keep
    CoreSim's model in agreement). DVE sums the strips at example
    finish. Weighted sums are interleaved into the NEXT example's score
    phase so the in-order PE never stalls at example boundaries; the
    LAST example instead weaves its mm4s into its OWN chunks via
    per-chunk exp of the just-finished score columns (kills the
    end-of-kernel serial tail).

Measured: 133.8us (previous session) -> ~122.6us (this session; 122.5/
122.7 back-to-back in a healthy device window), rel err 2.19e-3,
DMA-roofline-dominated (HBM read of fp32 x: ~94us min; DMA engines
active ~104us; Tensor busy ~103us). The chip intermittently enters a
~15%-slower sustained-load power state (~145us) that affects any build
equally.
"""

from contextlib import ExitStack

import numpy as np

import concourse.bass as bass
import concourse.tile as tile
from concourse import mybir
from concourse.bass_utils import run_bass_kernel_spmd

B, S, C, MID = 32, 8192, 256, 128
N_CORES = 8
B_LOC = B // N_CORES  # 4 examples per core

F32 = mybir.dt.float32
BF16 = mybir.dt.bfloat16
AF = mybir.ActivationFunctionType

S_TILES = S // 128  # 64 seq tiles of 128 per example
CH = 8  # chunks per example (1024 seq each; measured optimum -- both
# halving (CH=16) and doubling (CH=4) the DMA size cost 3-27us)
T_PER_CH = S_TILES // CH  # 8 seq tiles per chunk


def build_nc(reps: int = 1, strip_waits: bool = True) -> bass.Bass:
    """Build the per-core program. reps>1 repeats the whole computation
    back-to-back inside one NEFF (used only for benchmarking: the wall-time
    difference between reps=R and reps=1 isolates kernel time from dispatch
    overhead). strip_waits=False keeps Tile's full (redundant) semaphore
    waits -- required for CoreSim, whose race detector doesn't model
    engine-FIFO-implied ordering; hardware builds need the strip because
    walrus allows only one sync wait per engine ISA instruction."""
    nc = bass.Bass("TRN2", target_bir_lowering=False, debug=False)

    x_ext = nc.declare_dram_parameter("x", [B_LOC, S, C], F32, isOutput=False)
    # W1/w2 arrive host-pre-cast to bf16 (tiny constants) so their loads can
    # ride the Sync HWDGE ring -- only gpsimd DMAs can cast, and gpsimd must
    # stay free for the x-chunk pipeline from t=0.
    w1_ext = nc.declare_dram_parameter("W1", [C, MID], BF16, isOutput=False)
    b1_ext = nc.declare_dram_parameter("b1", [MID], F32, isOutput=False)
    w2_ext = nc.declare_dram_parameter("w2", [MID], BF16, isOutput=False)
    id_ext = nc.declare_dram_parameter("ident", [128, 128], BF16, isOutput=False)
    out_ext = nc.declare_dram_parameter("out", [B_LOC, C], F32, isOutput=True)

    with tile.TileContext(nc) as tc, ExitStack() as ctx:
        const_pool = ctx.enter_context(tc.tile_pool(name="const", bufs=1))
        # All 32 chunks resident (128KB/partition): no slot reuse means no
        # WAW pacing waits in the DMA queues -- the whole x load streams
        # back-to-back, so the E64 engine (which also serves the ~8us of
        # instruction-fetch DMAs) drains its share continuously instead of
        # crawling through a lone-engine backlog at the end of the load.
        xnat_pool = ctx.enter_context(tc.tile_pool(name="xnat", bufs=4 * CH))
        xt_pool = ctx.enter_context(tc.tile_pool(name="xt", bufs=3))
        ht_pool = ctx.enter_context(tc.tile_pool(name="ht", bufs=4))
        small_pool = ctx.enter_context(tc.tile_pool(name="small", bufs=4))
        psum_xt = ctx.enter_context(tc.tile_pool(name="psum_xt", bufs=1, space="PSUM"))
        psum_ht = ctx.enter_context(tc.tile_pool(name="psum_ht", bufs=2, space="PSUM"))
        psum_sc = ctx.enter_context(tc.tile_pool(name="psum_sc", bufs=2, space="PSUM"))
        psum_oa = ctx.enter_context(tc.tile_pool(name="psum_oa", bufs=2, space="PSUM"))

        # ---- HAM warm-up burst (no DMA dependencies: memset-fed) ----
        ones_bf = const_pool.tile([128, 1], BF16, tag="ones_bf")
        nc.vector.memset(ones_bf[:], 1.0)
        ones_sb = const_pool.tile([128, 1], F32, tag="ones")
        nc.vector.memset(ones_sb[:], 1.0)
        wburst_sb = const_pool.tile([128, 512], BF16, tag="wburst")
        nc.vector.memset(wburst_sb[:], 1.0)
        warm_ps = psum_ht.tile([128, 512], F32, tag="hT")
        for _ in range(16):
            nc.tensor.matmul(
                warm_ps[0:1, :], ones_bf[:], wburst_sb[:], start=True, stop=True
            )

        # ---- constants (one-time, tiny; Sync-engine HWDGE ring so they
        # don't queue behind x-chunk SWDGE descriptor generation) ----
        w1_sb = const_pool.tile([128, 2, MID], BF16, tag="w1")
        nc.sync.dma_start(
            out=w1_sb[:], in_=w1_ext[:].rearrange("(k p) m -> p k m", p=128)
        )
        w2_sb = const_pool.tile([128, 1], BF16, tag="w2")
        nc.sync.dma_start(out=w2_sb[:], in_=w2_ext[:].rearrange("(p o) -> p o", o=1))
        b1_sb = const_pool.tile([128, 1], F32, tag="b1")
        nc.sync.dma_start(out=b1_sb[:], in_=b1_ext[:].rearrange("(p o) -> p o", o=1))
        ident_sb = const_pool.tile([128, 128], BF16, tag="ident")
        nc.sync.dma_start(out=ident_sb[:], in_=id_ext[:])
        # Warm up the ScalarE activation table (exp_and_others: tanh+exp) on a
        # tiny input. Walrus attaches the table-load sync to the first
        # ACTIVATE; without this, that instruction exceeds the ISA's
        # 2-sync-wait budget once Tile's own deps are added.
        warm_sb = const_pool.tile([128, 1], F32, tag="warm")
        nc.scalar.activation(warm_sb[:], b1_sb[:], AF.Tanh, bias=b1_sb[:])
        # Warm the other engines too, and -- crucially -- make the PE observe
        # every constant's DMA-lane semaphore via one-wait warmup matmuls, so
        # no steady-state matmul ever needs a second (constant-load) wait.
        # Walrus enforces at most ONE sync wait per engine ISA instruction.
        warm2_sb = const_pool.tile([128, 1], F32, tag="warm2")
        nc.vector.tensor_copy(warm2_sb[:], ones_sb[:])
        nc.tensor.matmul(
            warm_ps[0:1, 0:1], ones_sb[:], ones_sb[:], start=True, stop=True
        )
        nc.tensor.matmul(
            warm_ps[0:1, 0:1], ident_sb[:, 0:1], ones_bf[:], start=True, stop=True
        )
        nc.tensor.matmul(
            warm_ps[0:1, 0:1], w1_sb[:, 0, 0:1], ones_bf[:], start=True, stop=True
        )
        nc.tensor.matmul(
            warm_ps[0:1, 0:1], ones_bf[:], w2_sb[:], start=True, stop=True
        )

        def emit_mm4_slice(st, lo, hi):
            """Accumulate seq tiles [lo, hi) of a pending example into the
            4 col-group strip accumulators (partitions 0/32/64/96)."""
            acc = st["acc"]
            for t in range(lo, hi):
                j = t % 4
                # skip_group_check: CoreSim's zero-region tracker flattens
                # base_partition!=0 PSUM APs to bogus (partition, bank) keys,
                # spuriously colliding with other pools. The strips write
                # disjoint partitions {0,32,64,96} of one bank; per-element
                # has_written accumulation is genuinely race-free.
                nc.tensor.matmul(
                    acc[32 * j : 32 * j + 1, :],
                    st["p_bf"][:, t : t + 1],
                    st["x_chunks"][t // T_PER_CH][:, t % T_PER_CH, :],
                    start=(t == 0),
                    stop=(t == S_TILES - 1),
                    tile_position=(0, 32 * j),
                    skip_group_check=True,
                )

        def finish_phase_b(st):
            # DVE reads at most ONE non-scalar operand from PSUM per
            # instruction (NCC_IBVF027): chain the strip adds through SBUF.
            acc = st["acc"]
            t0 = small_pool.tile([1, C], F32, tag="t0")
            nc.vector.tensor_copy(t0[:], acc[0:1, :])
            t1 = small_pool.tile([1, C], F32, tag="t1")
            nc.vector.tensor_tensor(t1[:], t0[:], acc[32:33, :], mybir.AluOpType.add)
            t2 = small_pool.tile([1, C], F32, tag="t2")
            nc.vector.tensor_tensor(t2[:], t1[:], acc[64:65, :], mybir.AluOpType.add)
            tsum = small_pool.tile([1, C], F32, tag="tsum")
            nc.vector.tensor_tensor(
                tsum[:], t2[:], acc[96:97, :], mybir.AluOpType.add
            )
            out_sb = small_pool.tile([1, C], F32, tag="out_sb")
            nc.vector.tensor_scalar_mul(out_sb[:], tsum[:], st["recip"][:])
            nc.scalar.dma_start(out=out_ext[st["b"] : st["b"] + 1, :], in_=out_sb[:])

        def emit_mm2(h_t, sc_ps, jj0):
            for j in range(4):
                nc.tensor.matmul(
                    sc_ps[:, jj0 + j : jj0 + j + 1],
                    h_t[:, j * 128 : (j + 1) * 128],
                    w2_sb[:],
                    start=True,
                    stop=True,
                )

        pending = None  # example whose weighted-sum mm4s are being woven
        mm2_defer = []  # at most one deferred (h_t, scoresT_ps, col) group
        blist = [bb for _ in range(reps) for bb in range(B_LOC)]
        for bi, b in enumerate(blist):
            is_last = bi == len(blist) - 1
            x_chunks = []
            # scoresT cols [0:64]; col [64] holds the cross-partition exp-sum.
            scoresT_ps = psum_sc.tile([128, S_TILES + 1], F32, tag="scoresT")
            # Dedicated accumulator bank: 4 col-group strip rows at partitions
            # {0,32,64,96}, ONE accumulation chain (start only at t=0 -- any
            # start=True clears has_written for the WHOLE bank, so per-strip
            # chains in a shared bank would destroy each other). Rows 32/64/96
            # are first written with start=False (has_written clear -> HW
            # overwrites); the DVE memsets below make CoreSim's accumulate-
            # based model agree with that overwrite semantics.
            acc_ps = psum_oa.tile([128, C], F32, tag="outacc")
            for j in (32, 64, 96):
                nc.vector.memset(acc_ps[j : j + 1, :], 0.0)
            if is_last:
                p_bf = small_pool.tile([128, S_TILES], BF16, tag="p")
                last_st = {"acc": acc_ps, "x_chunks": x_chunks, "b": b,
                           "p_bf": p_bf}
            for ch in range(CH):
                # keep the PE busy across example boundaries: weave the
                # previous example's weighted-sum matmuls between chunks.
                # Emitted at chunk TOP so later same-engine waits imply them.
                if pending is not None:
                    emit_mm4_slice(pending, ch * T_PER_CH, (ch + 1) * T_PER_CH)
                # load 1024 seq rows, cast fp32->bf16 during DMA (SWDGE --
                # the only casting path; gpsimd carries nothing else so
                # chunk 0's descriptors go out at t~0). Seq-tile relabeling:
                # tile tt = r holds rows s = p*8 + r, so each descriptor
                # covers 8 contiguous HBM rows (8KB source runs, one
                # descriptor per partition per chunk). Softmax/weighted-sum
                # are invariant to which rows form a tile as long as every
                # consumer derives from the same loaded layout.
                x_ch = xnat_pool.tile([128, T_PER_CH, C], BF16, tag="xnat")
                src = x_ext[b, ch * T_PER_CH * 128 : (ch + 1) * T_PER_CH * 128, :]
                rs = src.rearrange("(p r) c -> p r c", p=128, r=T_PER_CH)
                if bi == 0 and ch == 0:
                    # Split the pipeline-priming chunk into 4 sub-loads so the
                    # first transposes start after ~256KB instead of ~1MB.
                    # (Both a finer 128KB-first ladder and a coarser [2,6]
                    # split measured 3-5us WORSE: gen cost vs cold-BW
                    # granularity -- 4x2 tiles is the measured optimum.)
                    for rp in range(4):
                        nc.gpsimd.dma_start(
                            out=x_ch[:, 2 * rp : 2 * rp + 2, :],
                            in_=rs[:, 2 * rp : 2 * rp + 2, :],
                        )
                else:
                    nc.gpsimd.dma_start(out=x_ch[:], in_=rs)
                x_chunks.append(x_ch)

                # (7) TensorE transpose of each [128,128] block into bf16
                # PSUM, per c-half (one PSUM bank each); VectorE drains
                # PSUM -> SBUF in HALF-bank [128,512] copies issued as soon
                # as their 4 tiles are transposed, so mm1[s0]'s inputs are
                # ready before the PE finishes transposing tiles 4-7.
                ps_xt0 = psum_xt.tile([128, 1024], BF16, tag="ps_xt0")
                ps_xt1 = psum_xt.tile([128, 1024], BF16, tag="ps_xt1")
                xt0 = xt_pool.tile([128, 1024], BF16, tag="xt0")
                xt1 = xt_pool.tile([128, 1024], BF16, tag="xt1")
                for half in range(2):
                    for t in range(half * 4, half * 4 + 4):
                        nc.tensor.transpose(
                            ps_xt0[:, t * 128 : (t + 1) * 128],
                            x_ch[:, t, 0:128],
                            ident_sb[:],
                        )
                        nc.tensor.transpose(
                            ps_xt1[:, t * 128 : (t + 1) * 128],
                            x_ch[:, t, 128:256],
                            ident_sb[:],
                        )
                    lo, hi = half * 512, half * 512 + 512
                    nc.vector.tensor_copy(xt0[:, lo:hi], ps_xt0[:, lo:hi])
                    nc.vector.tensor_copy(xt1[:, lo:hi], ps_xt1[:, lo:hi])

                # mm1[s0]; the PREVIOUS chunk's deferred mm2[s1] group (its
                # tanh finished a chunk ago -- free PE work covering the
                # xt[512:] drain); mm1[s1]; tanhs; mm2[s0]; defer mm2[s1].
                h_ts = []
                for sub in range(2):
                    s0 = sub * 512
                    hT_ps = psum_ht.tile([128, 512], F32, tag="hT")
                    nc.tensor.matmul(
                        hT_ps[:], w1_sb[:, 0, :], xt0[:, s0 : s0 + 512],
                        start=True, stop=False,
                    )
                    nc.tensor.matmul(
                        hT_ps[:], w1_sb[:, 1, :], xt1[:, s0 : s0 + 512],
                        start=False, stop=True,
                    )
                    h_t = ht_pool.tile([128, 512], BF16, tag="ht")
                    nc.scalar.activation(h_t[:], hT_ps[:], AF.Tanh, bias=b1_sb[:])
                    h_ts.append(h_t)
                    if sub == 0 and mm2_defer:
                        emit_mm2(*mm2_defer.pop())
                for j in range(4):
                    jj = ch * T_PER_CH + j
                    nc.tensor.matmul(
                        scoresT_ps[:, jj : jj + 1],
                        h_ts[0][:, j * 128 : (j + 1) * 128],
                        w2_sb[:],
                        start=True,
                        stop=True,
                    )
                if is_last:
                    # the last example's tail weave needs the chunk's full
                    # scores immediately -- don't defer its s1 group
                    emit_mm2(h_ts[1], scoresT_ps, ch * T_PER_CH + 4)
                else:
                    mm2_defer.append(
                        (h_ts[1], scoresT_ps, ch * T_PER_CH + 4)
                    )

                if is_last:
                    # Tail weave: exp this chunk's just-finished score columns
                    # (no accum) and immediately emit its mm4s, so the last
                    # example's weighted sum overlaps its own score phase
                    # instead of serializing after it.
                    c0 = ch * T_PER_CH
                    nc.scalar.activation(
                        p_bf[:, c0 : c0 + T_PER_CH],
                        scoresT_ps[:, c0 : c0 + T_PER_CH],
                        AF.Exp,
                    )
                    emit_mm4_slice(last_st, c0, c0 + T_PER_CH)

                if pending is not None and ch == CH - 1:
                    finish_phase_b(pending)
                    pending = None

            # ---- this example's softmax reductions ----
            while mm2_defer:  # last chunk's deferred s1 group
                emit_mm2(*mm2_defer.pop())
            if is_last:
                # p_bf columns were produced per-chunk above; re-exp the full
                # scores into a throwaway tile just for the fused row-sum.
                p2_bf = small_pool.tile([128, S_TILES], BF16, tag="p2")
                sumrow = small_pool.tile([128, 1], F32, tag="sumrow")
                nc.scalar.activation(
                    p2_bf[:], scoresT_ps[:, 0:S_TILES], AF.Exp, accum_out=sumrow[:]
                )
            else:
                p_bf = small_pool.tile([128, S_TILES], BF16, tag="p")
                sumrow = small_pool.tile([128, 1], F32, tag="sumrow")
                nc.scalar.activation(
                    p_bf[:], scoresT_ps[:, 0:S_TILES], AF.Exp, accum_out=sumrow[:]
                )
            # cross-partition exp-sum lands in the scores tile's spare column
            # (same PSUM bank -> no extra slot, and the matmul's only wait is
            # the ScalarE accum above)
            nc.tensor.matmul(
                scoresT_ps[0:1, S_TILES : S_TILES + 1],
                ones_sb[:],
                sumrow[:],
                start=True,
                stop=True,
            )
            # give the DVE an up-to-date ScalarE observation so the
            # reciprocal's only explicit wait is the PE (sum matmul)
            dve_obs = small_pool.tile([1, 1], BF16, tag="dve_obs")
            nc.vector.tensor_copy(dve_obs[:], p_bf[0:1, 0:1] if not is_last else p2_bf[0:1, 0:1])
            recip = small_pool.tile([1, 1], F32, tag="recip")
            nc.vector.reciprocal(recip[:], scoresT_ps[0:1, S_TILES : S_TILES + 1])
            if is_last:
                last_st["recip"] = recip
                finish_phase_b(last_st)
            else:
                pending = {
                    "b": b,
                    "p_bf": p_bf,
                    "x_chunks": x_chunks,
                    "recip": recip,
                    "acc": acc_ps,
                }

    if strip_waits:
        _strip_implied_self_waits(nc)
    return nc


def _strip_implied_self_waits(nc: bass.Bass) -> None:
    """Reduce per-instruction sync waits to what the hardware needs.

    Walrus accepts at most ONE sync wait per engine ISA instruction, but
    Tile emits waits per logical dependency. Two sound reductions:

    1. Engine-clock elision. Each engine's sequencer evaluates waits in
       program order and engines retire in order, so if an earlier
       instruction on the SAME engine already waited for sem >= v' (v'>=v),
       a later instruction's wait for sem >= v is vacuous: the semaphore
       condition held before the predecessor issued. (Tile deliberately
       doesn't do this transitive per-proc minimization.) Also covers waits
       on the engine's own completion semaphore.

    2. x-load WAW elision. Each x-chunk load carries {PE >= k (WAR: all
       readers of the slot's old contents are done), DMASW >= v (WAW vs the
       old writer)}. The readers read-after-wrote the old data, so the WAR
       wait transitively dominates the WAW wait; drop the DMASW wait.
    """
    eng_prefix = {
        mybir.EngineType.PE: "PE_",
        mybir.EngineType.DVE: "DVE_",
        mybir.EngineType.Activation: "Activation_",
        mybir.EngineType.Pool: "Pool_",
        mybir.EngineType.SP: "SP_",
    }
    # Sems that are ever non-monotonically updated (barrier gather/release
    # use sem-sub) are excluded from all reasoning: their values regress.
    nonmono: set[str] = set()
    for f in nc.m.functions:
        for blk in f.blocks:
            for inst in blk.instructions:
                si = inst.sync_info
                if si is None:
                    continue
                for u in si.on_update:
                    if u.sync_type == "semaphore" and u.update_mode not in (
                        "sem-inc",
                        "sem-add-imm",
                    ):
                        nonmono.add(u.ant_name)

    observed: dict[mybir.EngineType, dict[str, int]] = {}
    for f in nc.m.functions:
        for blk in f.blocks:
            splits: list[tuple[int, list]] = []
            for idx, inst in enumerate(blk.instructions):
                si = inst.sync_info
                if si is None:
                    continue
                tn = type(inst).__name__
                if tn == "InstEventSemaphore":
                    continue  # barrier machinery: leave untouched
                eng = inst.engine
                obs = observed.setdefault(eng, {})
                pref = eng_prefix.get(eng)
                is_x_load = False
                if tn == "InstDMACopy" and eng == mybir.EngineType.Pool:
                    try:
                        is_x_load = "x_ch" in str(inst.outs[0])
                    except Exception:
                        is_x_load = False
                has_pe_wait = any(
                    w.sync_type == "semaphore" and w.ant_name.startswith("PE_")
                    for w in si.on_wait
                )
                kept = []
                for w in si.on_wait:
                    if (
                        w.sync_type != "semaphore"
                        or w.wait_mode != "sem-ge-imm"
                        or w.ant_name in nonmono
                        or tn == "InstDrain"
                    ):
                        kept.append(w)
                        continue
                    # (1) engine-clock / self-wait elision
                    if obs.get(w.ant_name, 0) >= w.wait_value:
                        continue
                    # (2) x-load WAW-vs-old-writer elision
                    if (
                        is_x_load
                        and has_pe_wait
                        and w.ant_name.startswith("DMASW")
                    ):
                        continue
                    kept.append(w)
                # record knowledge from ALL original waits (sound even for
                # stripped ones: the condition held at this program point)
                for w in si.on_wait:
                    if (
                        w.sync_type == "semaphore"
                        and w.wait_mode == "sem-ge-imm"
                        and w.ant_name not in nonmono
                    ):
                        if obs.get(w.ant_name, 0) < w.wait_value:
                            obs[w.ant_name] = w.wait_value
                if len(kept) != len(si.on_wait):
                    si.on_wait = kept
                    kept = si.on_wait  # re-read normalized
                if len(kept) > 1:
                    # Hardware takes one sync wait per instruction: carry the
                    # surplus on single-wait Drain instructions inserted just
                    # before (same engine => sequencer evaluates them first).
                    extras = []
                    for i, w in enumerate(kept[:-1]):
                        d = mybir.InstDrain(
                            name=f"{inst.name}-w{i}", ins=[], outs=[]
                        )
                        d.engine = inst.engine
                        d.sync_info = mybir.SyncInfo(on_wait=[w], on_update=[])
                        extras.append(d)
                    si.on_wait = [kept[-1]]
                    splits.append((idx, extras))
                # engine-own completion increments advance the engine clock.
                # Pool excluded: its 8 Q7 cores may retire out of order, so
                # completion-count knowledge is only valid for strict-FIFO
                # engines (wait-observation inheritance above is still valid
                # for Pool -- the NX sequencer evaluates waits in order).
                if pref is not None and eng != mybir.EngineType.Pool:
                    for u in si.on_update:
                        if (
                            u.sync_type == "semaphore"
                            and u.update_mode in ("sem-inc", "sem-add-imm")
                            and u.ant_name.startswith(pref)
                        ):
                            obs[u.ant_name] = obs.get(u.ant_name, 0) + (
                                u.update_value or 1
                            )
            if splits:
                il = blk.instructions
                for idx, extras in reversed(splits):
                    for d in reversed(extras):
                        il.insert(idx, d)


_NC_CACHE = None


def _get_nc() -> bass.Bass:
    global _NC_CACHE
    if _NC_CACHE is None:
        _NC_CACHE = build_nc()
    return _NC_CACHE


def kernel(x, W1, b1, w2, b2=None, **_unused) -> np.ndarray:
    """Full-input entry point: shard batch across 8 cores, run, gather.

    b2 is mathematically irrelevant (softmax shift invariance) and ignored.
    """
    x = np.ascontiguousarray(np.asarray(x, dtype=np.float32))
    W1 = np.ascontiguousarray(np.asarray(W1, dtype=np.float32))
    b1 = np.ascontiguousarray(np.asarray(b1, dtype=np.float32))
    w2 = np.ascontiguousarray(np.asarray(w2, dtype=np.float32))
    assert x.shape == (B, S, C), x.shape

    import ml_dtypes

    ident = np.eye(128, dtype=ml_dtypes.bfloat16)
    W1_bf = W1.astype(ml_dtypes.bfloat16)
    w2_bf = w2.astype(ml_dtypes.bfloat16)
    nc = _get_nc()
    in_maps = [
        {
            "x": x[i * B_LOC : (i + 1) * B_LOC],
            "W1": W1_bf,
            "b1": b1,
            "w2": w2_bf,
            "ident": ident,
        }
        for i in range(N_CORES)
    ]
    res = run_bass_kernel_spmd(nc, in_maps, list(range(N_CORES))).results
    out = np.concatenate([res[i]["out"] for i in range(N_CORES)], axis=0)
    return out.astype(np.float32)

